# revision 21
# baseline (speedup 1.0000x reference)
"""Conv2d 3x3 s1 p1 kernel for Trainium2, 8 NeuronCores.

Problem: x [32, 128, 56, 56] f32, weight [256, 128, 3, 3] f32 (OIHW)
         -> out [32, 256, 56, 56] f32  (stride 1, pad 1, no bias)

Strategy:
  - Data-parallel over batch: 4 images per core, 8 cores.
  - Conv expressed as 9 shifted matmuls accumulated in fp32 PSUM:
      out[co, p] += W[ky,kx][ci, co].T @ x[ci, p_shifted]
    with C_in = 128 exactly filling the contraction (partition) dim.
  - Host pre-pads width 56 -> 58 with zero columns so every horizontal
    tap is a plain strided window; vertical taps are handled by clipping
    whole rows at the first/last row-block (psum stays contiguous).
  - Host pre-transposes weight OIHW -> [ci, tap, co] so the stationary
    operand DMAs contiguously.
  - Inputs are shipped as fp16 (psum accumulates fp32): full PE rate,
    fast weight load, half the input DMA bytes. End-to-end rel err vs
    the fp32 reference is ~3e-4.
  - Each image is loaded as three overlapping row-chunks, loads
    dependency-chained so the first chunk lands first and compute starts
    as early as possible; a short burst of dummy matmuls during the load
    window lifts the PE out of its cold clock-gate state.
"""

import sys

if "/opt/trn_rl_repo" not in sys.path:
    sys.path.insert(0, "/opt/trn_rl_repo")

import numpy as np

N_CORES = 8
N_PER = 4          # images per core
CIN = 128
COUT = 256
H = W = 56
WP = 58            # padded width
RB = 8             # output rows per block
NBLK = H // RB     # 7 row blocks per image
NFREE = RB * W     # 448 psum columns per block

# image row-chunks: chunk c holds global rows [starts[c], ends[c]).
# image 0 uses finer chunks so the first row block can start as soon as
# possible after the ~2us DMA completion latency.
CHUNKS = [
    [(0, 10), (7, 17), (15, 33), (31, 56)],   # image 0
    [(0, 17), (15, 33), (31, 56)],
    [(0, 17), (15, 33), (31, 56)],
    [(0, 17), (15, 33), (31, 56)],
]
BLK_CHUNK = [
    [0, 1, 2, 2, 3, 3, 3],              # image 0: which chunk per row block
    [0, 0, 1, 1, 2, 2, 2],
    [0, 0, 1, 1, 2, 2, 2],
    [0, 0, 1, 1, 2, 2, 2],
]

N_WARM = 7         # dummy matmuls to exit the cold PE clock-gate state

# taps ordered so the first (dy=0) always covers the full row block;
# clipped taps then accumulate onto an initialized psum range.
TAPS = [(0, 0), (0, 1), (0, 2),
        (-1, 0), (-1, 1), (-1, 2),
        (1, 0), (1, 1), (1, 2)]

_cache = {}


def _build():
    import concourse.bass as bass  # noqa: F401
    import concourse.mybir as mybir
    import concourse.tile as tile
    from concourse import bacc
    from concourse.tile_rust import add_dep_helper

    nc = bacc.Bacc("TRN2", target_bir_lowering=False, debug=False,
                   num_devices=N_CORES)
    xd = nc.dram_tensor("x", [N_PER, CIN, H, WP], mybir.dt.float16,
                        kind="ExternalInput")
    wd = nc.dram_tensor("wt", [2, CIN, 9, 128], mybir.dt.float16,
                        kind="ExternalInput")
    yd = nc.dram_tensor("y", [N_PER, COUT, H * W], mybir.dt.float32,
                        kind="ExternalOutput")

    with tile.TileContext(nc) as tc:
        with tc.tile_pool(name="wpool", bufs=1) as wpool, \
             tc.tile_pool(name="xpool", bufs=1) as xpool, \
             tc.tile_pool(name="spool", bufs=3) as spool, \
             tc.tile_pool(name="pspool", bufs=4, space="PSUM") as pspool, \
             tc.tile_pool(name="warmp", bufs=1, space="PSUM") as warmp:

            # PE warm-up: a burst of throwaway matmuls with no DMA deps so
            # the PE's activity monitor releases the clock gate while the
            # first input chunks are still in flight.
            dummy = wpool.tile([CIN, NFREE], mybir.dt.float16, tag="dummy")
            nc.vector.memset(dummy[:], 0.0)
            wps = warmp.tile([128, NFREE], mybir.dt.float32)
            for i in range(N_WARM):
                nc.tensor.matmul(wps[:], dummy[:, :128], dummy[:],
                                 start=True, stop=True)

            # all input loads are chained so they complete in program order
            # (SDMA otherwise round-robins all queues and everything lands
            # at the same late time); the chain is ordered by when compute
            # first needs each piece.
            prev = None

            def chained_dma(out, in_):
                nonlocal prev
                dma = nc.sync.dma_start(out=out, in_=in_)
                if prev is not None:
                    add_dep_helper(dma.ins, prev.ins, sync=True,
                                   reason="serialize input loads")
                prev = dma

            wts = [wpool.tile([CIN, 9, 128], mybir.dt.float16,
                              name=f"w{cb}", tag=f"w{cb}")
                   for cb in range(2)]
            xts = [[xpool.tile([CIN, r1 - r0, WP], mybir.dt.float16,
                               name=f"x{n}c{c}", tag=f"x{n}c{c}")
                    for c, (r0, r1) in enumerate(CHUNKS[n])]
                   for n in range(N_PER)]

            # wt0 rides its own lane in parallel with the head of the x chain
            nc.sync.dma_start(out=wts[0][:], in_=wd.ap()[0])
            for c, (r0, r1) in enumerate(CHUNKS[0]):
                chained_dma(xts[0][c][:], xd.ap()[0][:, r0:r1, :])
            chained_dma(wts[1][:], wd.ap()[1])
            for n in range(1, N_PER):
                for c, (r0, r1) in enumerate(CHUNKS[n]):
                    chained_dma(xts[n][c][:], xd.ap()[n][:, r0:r1, :])

            for n in range(N_PER):
                for cb in range(2):
                    stage = spool.tile([128, H * W], mybir.dt.float32)
                    for blk in range(NBLK):
                        h0 = blk * RB
                        c = BLK_CHUNK[n][blk]
                        roff = CHUNKS[n][c][0]
                        ps = pspool.tile([128, NFREE], mybir.dt.float32)
                        for i, (dy, kx) in enumerate(TAPS):
                            lo = max(0, h0 + dy)
                            hi = min(H - 1, h0 + RB - 1 + dy)
                            out_off = (lo - dy - h0) * W
                            nc.tensor.matmul(
                                ps[:, out_off:out_off + (hi - lo + 1) * W],
                                wts[cb][:, (dy + 1) * 3 + kx, :],
                                xts[n][c][:, lo - roff:hi - roff + 1,
                                          kx:kx + W],
                                start=(i == 0), stop=(i == len(TAPS) - 1),
                            )
                        last_stage = (n == N_PER - 1 and cb == 1)
                        if last_stage and blk == NBLK - 1:
                            # final block: copy+flush in two halves so the
                            # post-last-matmul critical path is as short as
                            # possible (each DMA completion costs ~2us)
                            hf = NFREE // 2
                            for h in range(2):
                                nc.vector.tensor_copy(
                                    out=stage[:, blk * NFREE + h * hf:
                                              blk * NFREE + (h + 1) * hf],
                                    in_=ps[:, h * hf:(h + 1) * hf],
                                )
                                nc.sync.dma_start(
                                    out=yd.ap()[n, cb * 128:(cb + 1) * 128,
                                                blk * NFREE + h * hf:
                                                blk * NFREE + (h + 1) * hf],
                                    in_=stage[:, blk * NFREE + h * hf:
                                              blk * NFREE + (h + 1) * hf],
                                )
                            continue
                        nc.vector.tensor_copy(
                            out=stage[:, blk * NFREE:(blk + 1) * NFREE],
                            in_=ps[:],
                        )
                        # flush staged rows as they complete so the final
                        # store isn't one big DMA serialized after the last
                        # matmul; the very last stage flushes every block
                        if last_stage:
                            # flush eagerly so the post-last-matmul store is
                            # a single small block on the FIFO'd HWDGE ring
                            if blk == 1:
                                nc.sync.dma_start(
                                    out=yd.ap()[n, cb * 128:(cb + 1) * 128,
                                                :2 * NFREE],
                                    in_=stage[:, :2 * NFREE],
                                )
                            elif blk >= 2:
                                nc.sync.dma_start(
                                    out=yd.ap()[n, cb * 128:(cb + 1) * 128,
                                                blk * NFREE:(blk + 1) * NFREE],
                                    in_=stage[:, blk * NFREE:(blk + 1) * NFREE],
                                )
                        elif blk in (1, 3, 5):
                            nc.sync.dma_start(
                                out=yd.ap()[n, cb * 128:(cb + 1) * 128,
                                            (blk - 1) * NFREE:
                                            (blk + 1) * NFREE],
                                in_=stage[:, (blk - 1) * NFREE:
                                          (blk + 1) * NFREE],
                            )
                    if not last_stage:
                        nc.sync.dma_start(
                            out=yd.ap()[n, cb * 128:(cb + 1) * 128,
                                        6 * NFREE:],
                            in_=stage[:, 6 * NFREE:],
                        )

    nc.compile()
    return nc


def _get_nc():
    if "nc" not in _cache:
        _cache["nc"] = _build()
    return _cache["nc"]


def _run(x, weight, trace=False):
    from concourse.bass_utils import run_bass_kernel_spmd

    nc = _get_nc()

    x = np.ascontiguousarray(x, dtype=np.float32)
    weight = np.ascontiguousarray(weight, dtype=np.float32)

    # pad width with one zero column on each side; ship as fp16
    xp = np.zeros((32, CIN, H, WP), dtype=np.float16)
    xp[:, :, :, 1:1 + W] = x.astype(np.float16)

    # OIHW -> [co_half, ci, tap(ky*3+kx), co%128], contiguous per ci row
    wt = np.ascontiguousarray(
        weight.reshape(2, 128, CIN, 9).transpose(0, 2, 3, 1)
    ).astype(np.float16)

    in_maps = [
        {"x": xp[c * N_PER:(c + 1) * N_PER], "wt": wt}
        for c in range(N_CORES)
    ]
    res = run_bass_kernel_spmd(nc, in_maps, core_ids=list(range(N_CORES)),
                               trace=trace)
    out = np.concatenate(
        [res.results[c]["y"].reshape(N_PER, COUT, H, W) for c in range(N_CORES)],
        axis=0,
    )
    return out, res


def kernel(x, weight):
    out, _ = _run(x, weight, trace=False)
    return out


# revision 22
# speedup vs baseline: 1.0229x; 1.0229x over previous
"""Conv2d 3x3 s1 p1 kernel for Trainium2, 8 NeuronCores.

Problem: x [32, 128, 56, 56] f32, weight [256, 128, 3, 3] f32 (OIHW)
         -> out [32, 256, 56, 56] f32  (stride 1, pad 1, no bias)

Strategy:
  - Data-parallel over batch: 4 images per core, 8 cores.
  - Conv expressed as 9 shifted matmuls accumulated in fp32 PSUM:
      out[co, p] += W[ky,kx][ci, co].T @ x[ci, p_shifted]
    with C_in = 128 exactly filling the contraction (partition) dim.
  - Host pre-pads width 56 -> 58 with zero columns so every horizontal
    tap is a plain strided window; vertical taps are handled by clipping
    whole rows at the first/last row-block (psum stays contiguous).
  - Host pre-transposes weight OIHW -> [ci, tap, co] so the stationary
    operand DMAs contiguously.
  - Inputs are shipped as fp16 (psum accumulates fp32): full PE rate,
    fast weight load, half the input DMA bytes. End-to-end rel err vs
    the fp32 reference is ~3e-4.
  - Each image is loaded as three overlapping row-chunks, loads
    dependency-chained so the first chunk lands first and compute starts
    as early as possible; a short burst of dummy matmuls during the load
    window lifts the PE out of its cold clock-gate state.
"""

import sys

if "/opt/trn_rl_repo" not in sys.path:
    sys.path.insert(0, "/opt/trn_rl_repo")

import numpy as np

N_CORES = 8
N_PER = 4          # images per core
CIN = 128
COUT = 256
H = W = 56
WP = 58            # padded width
RB = 8             # output rows per block
NBLK = H // RB     # 7 row blocks per image
NFREE = RB * W     # 448 psum columns per block

# image row-chunks: chunk c holds global rows [starts[c], ends[c])
CHUNKS = [[(0, 17), (15, 33), (31, 56)]] * 4
BLK_CHUNK = [[0, 0, 1, 1, 2, 2, 2]] * 4   # which chunk serves each row block

N_WARM = 7         # dummy matmuls to exit the cold PE clock-gate state

# taps ordered so the first (dy=0) always covers the full row block;
# clipped taps then accumulate onto an initialized psum range.
TAPS = [(0, 0), (0, 1), (0, 2),
        (-1, 0), (-1, 1), (-1, 2),
        (1, 0), (1, 1), (1, 2)]

_cache = {}


def _build():
    import concourse.bass as bass  # noqa: F401
    import concourse.mybir as mybir
    import concourse.tile as tile
    from concourse import bacc
    from concourse.tile_rust import add_dep_helper

    nc = bacc.Bacc("TRN2", target_bir_lowering=False, debug=False,
                   num_devices=N_CORES)
    xd = nc.dram_tensor("x", [N_PER, CIN, H, WP], mybir.dt.float16,
                        kind="ExternalInput")
    wd = nc.dram_tensor("wt", [2, CIN, 9, 128], mybir.dt.float16,
                        kind="ExternalInput")
    yd = nc.dram_tensor("y", [N_PER, COUT, H * W], mybir.dt.float32,
                        kind="ExternalOutput")

    with tile.TileContext(nc) as tc:
        with tc.tile_pool(name="wpool", bufs=1) as wpool, \
             tc.tile_pool(name="xpool", bufs=1) as xpool, \
             tc.tile_pool(name="spool", bufs=3) as spool, \
             tc.tile_pool(name="pspool", bufs=4, space="PSUM") as pspool, \
             tc.tile_pool(name="warmp", bufs=1, space="PSUM") as warmp:

            # PE warm-up: a burst of throwaway matmuls with no DMA deps so
            # the PE's activity monitor releases the clock gate while the
            # first input chunks are still in flight.
            dummy = wpool.tile([CIN, NFREE], mybir.dt.float16, tag="dummy")
            nc.vector.memset(dummy[:], 0.0)
            wps = warmp.tile([128, NFREE], mybir.dt.float32)
            for i in range(N_WARM):
                nc.tensor.matmul(wps[:], dummy[:, :128], dummy[:],
                                 start=True, stop=True)

            # all input loads are chained so they complete in program order
            # (SDMA otherwise round-robins all queues and everything lands
            # at the same late time); the chain is ordered by when compute
            # first needs each piece.
            prev = None

            def chained_dma(out, in_):
                nonlocal prev
                dma = nc.sync.dma_start(out=out, in_=in_)
                if prev is not None:
                    add_dep_helper(dma.ins, prev.ins, sync=True,
                                   reason="serialize input loads")
                prev = dma

            wts = [wpool.tile([CIN, 9, 128], mybir.dt.float16,
                              name=f"w{cb}", tag=f"w{cb}")
                   for cb in range(2)]
            xts = [[xpool.tile([CIN, r1 - r0, WP], mybir.dt.float16,
                               name=f"x{n}c{c}", tag=f"x{n}c{c}")
                    for c, (r0, r1) in enumerate(CHUNKS[n])]
                   for n in range(N_PER)]

            # wt0 rides its own lane in parallel with the head of the x chain
            nc.sync.dma_start(out=wts[0][:], in_=wd.ap()[0])
            for c, (r0, r1) in enumerate(CHUNKS[0]):
                chained_dma(xts[0][c][:], xd.ap()[0][:, r0:r1, :])
            chained_dma(wts[1][:], wd.ap()[1])
            for n in range(1, N_PER):
                for c, (r0, r1) in enumerate(CHUNKS[n]):
                    chained_dma(xts[n][c][:], xd.ap()[n][:, r0:r1, :])

            for n in range(N_PER):
                for cb in range(2):
                    stage = spool.tile([128, H * W], mybir.dt.float32)
                    for blk in range(NBLK):
                        h0 = blk * RB
                        c = BLK_CHUNK[n][blk]
                        roff = CHUNKS[n][c][0]
                        ps = pspool.tile([128, NFREE], mybir.dt.float32)
                        for i, (dy, kx) in enumerate(TAPS):
                            lo = max(0, h0 + dy)
                            hi = min(H - 1, h0 + RB - 1 + dy)
                            out_off = (lo - dy - h0) * W
                            nc.tensor.matmul(
                                ps[:, out_off:out_off + (hi - lo + 1) * W],
                                wts[cb][:, (dy + 1) * 3 + kx, :],
                                xts[n][c][:, lo - roff:hi - roff + 1,
                                          kx:kx + W],
                                start=(i == 0), stop=(i == len(TAPS) - 1),
                            )
                        last_stage = (n == N_PER - 1 and cb == 1)
                        if last_stage and blk == NBLK - 1:
                            # final block: copy+flush in two halves so the
                            # post-last-matmul critical path is as short as
                            # possible (each DMA completion costs ~2us)
                            hf = NFREE // 2
                            for h in range(2):
                                nc.vector.tensor_copy(
                                    out=stage[:, blk * NFREE + h * hf:
                                              blk * NFREE + (h + 1) * hf],
                                    in_=ps[:, h * hf:(h + 1) * hf],
                                )
                                nc.sync.dma_start(
                                    out=yd.ap()[n, cb * 128:(cb + 1) * 128,
                                                blk * NFREE + h * hf:
                                                blk * NFREE + (h + 1) * hf],
                                    in_=stage[:, blk * NFREE + h * hf:
                                              blk * NFREE + (h + 1) * hf],
                                )
                            continue
                        nc.vector.tensor_copy(
                            out=stage[:, blk * NFREE:(blk + 1) * NFREE],
                            in_=ps[:],
                        )
                        # flush staged rows as they complete so the final
                        # store isn't one big DMA serialized after the last
                        # matmul; the very last stage flushes every block
                        if last_stage:
                            # flush eagerly so the post-last-matmul store is
                            # a single small block on the FIFO'd HWDGE ring
                            if blk == 1:
                                nc.sync.dma_start(
                                    out=yd.ap()[n, cb * 128:(cb + 1) * 128,
                                                :2 * NFREE],
                                    in_=stage[:, :2 * NFREE],
                                )
                            elif blk >= 2:
                                nc.sync.dma_start(
                                    out=yd.ap()[n, cb * 128:(cb + 1) * 128,
                                                blk * NFREE:(blk + 1) * NFREE],
                                    in_=stage[:, blk * NFREE:(blk + 1) * NFREE],
                                )
                        elif blk in (1, 3, 5):
                            nc.sync.dma_start(
                                out=yd.ap()[n, cb * 128:(cb + 1) * 128,
                                            (blk - 1) * NFREE:
                                            (blk + 1) * NFREE],
                                in_=stage[:, (blk - 1) * NFREE:
                                          (blk + 1) * NFREE],
                            )
                    if not last_stage:
                        nc.sync.dma_start(
                            out=yd.ap()[n, cb * 128:(cb + 1) * 128,
                                        6 * NFREE:],
                            in_=stage[:, 6 * NFREE:],
                        )

    nc.compile()
    return nc


def _get_nc():
    if "nc" not in _cache:
        _cache["nc"] = _build()
    return _cache["nc"]


def _run(x, weight, trace=False):
    from concourse.bass_utils import run_bass_kernel_spmd

    nc = _get_nc()

    x = np.ascontiguousarray(x, dtype=np.float32)
    weight = np.ascontiguousarray(weight, dtype=np.float32)

    # pad width with one zero column on each side; ship as fp16
    xp = np.zeros((32, CIN, H, WP), dtype=np.float16)
    xp[:, :, :, 1:1 + W] = x.astype(np.float16)

    # OIHW -> [co_half, ci, tap(ky*3+kx), co%128], contiguous per ci row
    wt = np.ascontiguousarray(
        weight.reshape(2, 128, CIN, 9).transpose(0, 2, 3, 1)
    ).astype(np.float16)

    in_maps = [
        {"x": xp[c * N_PER:(c + 1) * N_PER], "wt": wt}
        for c in range(N_CORES)
    ]
    res = run_bass_kernel_spmd(nc, in_maps, core_ids=list(range(N_CORES)),
                               trace=trace)
    out = np.concatenate(
        [res.results[c]["y"].reshape(N_PER, COUT, H, W) for c in range(N_CORES)],
        axis=0,
    )
    return out, res


def kernel(x, weight):
    out, _ = _run(x, weight, trace=False)
    return out
